# revision 6
# baseline (speedup 1.0000x reference)
"""Biquad lowpass filter (torchaudio-style) as a Trainium2 Bass kernel.

Math: the reference IIR y[t] = b0 x[t] + b1 x[t-1] + b2 x[t-2] - a1 y[t-1]
- a2 y[t-2] has complex poles of radius sqrt(a2) ~= 0.4203, so its impulse
response decays below 1e-12 (relative, L2) within 32 taps. With zero
initial state the output is exactly y = conv(x, h) for the truncated
impulse response h[0..31] up to fp32 rounding.

Layout: per core (8 cores x 16 batch rows), each row's 160000 samples are
blocked into 1250 columns of 128 consecutive samples (host-side
transpose). The convolution is then a blocked-Toeplitz matmul on the
TensorEngine: out column j needs W0 @ x_col[j] + W1 @ x_col[j-1], where
W0[c, p] = h[p-c] (lower-triangular band) and W1[c, p] = h[p-c+128]
(upper-right corner, taps crossing the 128-block boundary). A zero column
is prepended per row so j=0 sees zero history (zero initial state).

Precision: inputs are split hi/lo into two bf16 streams on the host
(x = xh + xl, w = wh + wl), and each Toeplitz product is computed as
wh@xh + wl@xh + wh@xl (dropping the lo*lo term) with fp32 PSUM
accumulation — near-fp32 accuracy (~5e-6 rel) at full bf16 PE rate,
with the same DMA byte volume as one fp32 stream.
"""

import math

import numpy as np
import ml_dtypes

import concourse.bacc as bacc
import concourse.mybir as mybir
from concourse.tile import TileContext
from concourse import bass_utils

N_CORES = 8
B, T = 128, 160000
ROWS_PER_CORE = B // N_CORES  # 16
BLK = 128                     # samples per Toeplitz block (= partition dim)
NBLK = T // BLK               # 1250 blocks per row
K_TAPS = 32                   # truncated impulse response length
COLS_IN = NBLK + 1            # +1 zero history column per row
# PSUM output free-dim chunks (one fp32 PSUM bank = 512)
CHUNKS = [(0, 418), (418, 834), (834, NBLK)]

SAMPLE_RATE = 44100.0
CUTOFF = 10000.0
Q_VAL = 0.707


def _impulse_response(k: int) -> np.ndarray:
    w0 = 2.0 * math.pi * CUTOFF / SAMPLE_RATE
    alpha = math.sin(w0) / (2.0 * Q_VAL)
    cos_w0 = math.cos(w0)
    a0 = 1.0 + alpha
    b0, b1, b2 = (1.0 - cos_w0) / 2.0 / a0, (1.0 - cos_w0) / a0, (1.0 - cos_w0) / 2.0 / a0
    a1, a2 = -2.0 * cos_w0 / a0, (1.0 - alpha) / a0
    h = np.zeros(k, dtype=np.float64)
    y1 = y2 = x1 = x2 = 0.0
    for t in range(k):
        xt = 1.0 if t == 0 else 0.0
        y = b0 * xt + b1 * x1 + b2 * x2 - a1 * y1 - a2 * y2
        h[t] = y
        y2, y1 = y1, y
        x2, x1 = x1, xt
    return h


def _split_bf16(v: np.ndarray):
    hi = v.astype(ml_dtypes.bfloat16)
    lo = (v.astype(np.float32) - hi.astype(np.float32)).astype(ml_dtypes.bfloat16)
    return hi, lo


def _weights() -> np.ndarray:
    """[128, 512] bf16 = [W0h | W1h | W0l | W1l].

    W0[c, p] = h[p-c], W1[c, p] = h[p-c+128], each split hi/lo."""
    h = _impulse_response(K_TAPS)
    c = np.arange(BLK)[:, None]
    p = np.arange(BLK)[None, :]
    k0 = p - c
    k1 = p - c + BLK
    w0 = np.where((k0 >= 0) & (k0 < K_TAPS), h[np.clip(k0, 0, K_TAPS - 1)], 0.0)
    w1 = np.where((k1 >= 0) & (k1 < K_TAPS), h[np.clip(k1, 0, K_TAPS - 1)], 0.0)
    w0h, w0l = _split_bf16(w0.astype(np.float32))
    w1h, w1l = _split_bf16(w1.astype(np.float32))
    return np.concatenate([w0h, w1h, w0l, w1l], axis=1)


_COMPILED = None


def _build():
    nc = bacc.Bacc("TRN2", target_bir_lowering=False, debug=False,
                   num_devices=N_CORES)
    f32 = mybir.dt.float32
    bf16 = mybir.dt.bfloat16
    xh = nc.declare_dram_parameter("xh", [BLK, ROWS_PER_CORE * COLS_IN], bf16,
                                   isOutput=False)
    xl = nc.declare_dram_parameter("xl", [BLK, ROWS_PER_CORE * COLS_IN], bf16,
                                   isOutput=False)
    w = nc.declare_dram_parameter("w", [BLK, 4 * BLK], bf16, isOutput=False)
    out = nc.declare_dram_parameter("out", [BLK, ROWS_PER_CORE * NBLK], f32,
                                    isOutput=True)

    with TileContext(nc) as tc:
        with (
            tc.tile_pool(name="wpool", bufs=1) as wpool,
            tc.tile_pool(name="xpool", bufs=3) as xpool,
            tc.tile_pool(name="opool", bufs=3) as opool,
            tc.tile_pool(name="pspool", bufs=4, space="PSUM") as pspool,
        ):
            wt = wpool.tile([BLK, 4 * BLK], bf16)
            nc.sync.dma_start(wt[:], w[:])
            w0h, w1h = wt[:, 0:BLK], wt[:, BLK:2 * BLK]
            w0l, w1l = wt[:, 2 * BLK:3 * BLK], wt[:, 3 * BLK:4 * BLK]
            for r in range(ROWS_PER_CORE):
                xht = xpool.tile([BLK, COLS_IN], bf16, tag="xh")
                xlt = xpool.tile([BLK, COLS_IN], bf16, tag="xl")
                nc.sync.dma_start(xht[:], xh[:, r * COLS_IN:(r + 1) * COLS_IN])
                nc.sync.dma_start(xlt[:], xl[:, r * COLS_IN:(r + 1) * COLS_IN])
                ot = opool.tile([BLK, NBLK], f32)
                for j0, j1 in CHUNKS:
                    ps = pspool.tile([BLK, 512], f32)
                    n = j1 - j0
                    cur_h = xht[:, 1 + j0:1 + j1]
                    prv_h = xht[:, j0:j1]
                    cur_l = xlt[:, 1 + j0:1 + j1]
                    prv_l = xlt[:, j0:j1]
                    nc.tensor.matmul(ps[:, :n], w0h, cur_h, start=True, stop=False)
                    nc.tensor.matmul(ps[:, :n], w1h, prv_h, start=False, stop=False)
                    nc.tensor.matmul(ps[:, :n], w0l, cur_h, start=False, stop=False)
                    nc.tensor.matmul(ps[:, :n], w1l, prv_h, start=False, stop=False)
                    nc.tensor.matmul(ps[:, :n], w0h, cur_l, start=False, stop=False)
                    nc.tensor.matmul(ps[:, :n], w1h, prv_l, start=False, stop=True)
                    nc.vector.tensor_copy(ot[:, j0:j1], ps[:, :n])
                nc.sync.dma_start(out[:, r * NBLK:(r + 1) * NBLK], ot[:])
    nc.compile()
    return nc


def _get_compiled():
    global _COMPILED
    if _COMPILED is None:
        _COMPILED = _build()
    return _COMPILED


def kernel(clip: np.ndarray) -> np.ndarray:
    clip = np.ascontiguousarray(clip, dtype=np.float32)
    assert clip.shape == (B, T)
    w_np = _weights()
    in_maps = []
    for core in range(N_CORES):
        rows = clip[core * ROWS_PER_CORE:(core + 1) * ROWS_PER_CORE]
        # [16, 160000] -> [16, 1250, 128] -> per-row transpose [16, 128, 1250]
        blocks = rows.reshape(ROWS_PER_CORE, NBLK, BLK).transpose(0, 2, 1)
        xc = np.zeros((BLK, ROWS_PER_CORE, COLS_IN), dtype=np.float32)
        xc[:, :, 1:] = blocks.transpose(1, 0, 2)
        xc = xc.reshape(BLK, ROWS_PER_CORE * COLS_IN)
        xh_np, xl_np = _split_bf16(xc)
        in_maps.append({"xh": np.ascontiguousarray(xh_np),
                        "xl": np.ascontiguousarray(xl_np),
                        "w": w_np})

    nc = _get_compiled()
    res = bass_utils.run_bass_kernel_spmd(
        nc, in_maps, core_ids=list(range(N_CORES)))

    out = np.empty((B, T), dtype=np.float32)
    for core in range(N_CORES):
        yc = res.results[core]["out"]  # [128, 16*1250]
        yc = yc.reshape(BLK, ROWS_PER_CORE, NBLK).transpose(1, 2, 0)
        out[core * ROWS_PER_CORE:(core + 1) * ROWS_PER_CORE] = yc.reshape(
            ROWS_PER_CORE, T)
    return out


# revision 7
# speedup vs baseline: 1.0180x; 1.0180x over previous
"""Biquad lowpass filter (torchaudio-style) as a Trainium2 Bass kernel.

Math: the reference IIR y[t] = b0 x[t] + b1 x[t-1] + b2 x[t-2] - a1 y[t-1]
- a2 y[t-2] has complex poles of radius sqrt(a2) ~= 0.4203, so its impulse
response decays below 1e-12 (relative, L2) within 32 taps. With zero
initial state the output is exactly y = conv(x, h) for the truncated
impulse response h[0..31] up to fp32 rounding.

Layout: per core (8 cores x 16 batch rows), each row's 160000 samples are
blocked into 1250 columns of 128 consecutive samples (host-side
transpose). The convolution is then a blocked-Toeplitz matmul on the
TensorEngine: out column j needs W0 @ x_col[j] + W1 @ x_col[j-1], where
W0[c, p] = h[p-c] (lower-triangular band) and W1[c, p] = h[p-c+128]
(upper-right corner, taps crossing the 128-block boundary). A zero column
is prepended per row so j=0 sees zero history (zero initial state).

Precision: inputs are split hi/lo into two bf16 streams on the host
(x = xh + xl, w = wh + wl), and each Toeplitz product is computed as
wh@xh + wl@xh + wh@xl (dropping the lo*lo term) with fp32 PSUM
accumulation — near-fp32 accuracy (~5e-6 rel) at full bf16 PE rate,
with the same DMA byte volume as one fp32 stream.
"""

import math

import numpy as np
import ml_dtypes

import concourse.bacc as bacc
import concourse.mybir as mybir
from concourse.tile import TileContext
from concourse import bass_utils

N_CORES = 8
B, T = 128, 160000
ROWS_PER_CORE = B // N_CORES  # 16
BLK = 128                     # samples per Toeplitz block (= partition dim)
NBLK = T // BLK               # 1250 blocks per row
K_TAPS = 32                   # truncated impulse response length
COLS_IN = NBLK + 1            # +1 zero history column per row
# PSUM output free-dim chunks (one fp32 PSUM bank = 512)
CHUNKS = [(0, 418), (418, 834), (834, NBLK)]

SAMPLE_RATE = 44100.0
CUTOFF = 10000.0
Q_VAL = 0.707


def _impulse_response(k: int) -> np.ndarray:
    w0 = 2.0 * math.pi * CUTOFF / SAMPLE_RATE
    alpha = math.sin(w0) / (2.0 * Q_VAL)
    cos_w0 = math.cos(w0)
    a0 = 1.0 + alpha
    b0, b1, b2 = (1.0 - cos_w0) / 2.0 / a0, (1.0 - cos_w0) / a0, (1.0 - cos_w0) / 2.0 / a0
    a1, a2 = -2.0 * cos_w0 / a0, (1.0 - alpha) / a0
    h = np.zeros(k, dtype=np.float64)
    y1 = y2 = x1 = x2 = 0.0
    for t in range(k):
        xt = 1.0 if t == 0 else 0.0
        y = b0 * xt + b1 * x1 + b2 * x2 - a1 * y1 - a2 * y2
        h[t] = y
        y2, y1 = y1, y
        x2, x1 = x1, xt
    return h


def _split_bf16(v: np.ndarray):
    hi = v.astype(ml_dtypes.bfloat16)
    lo = (v.astype(np.float32) - hi.astype(np.float32)).astype(ml_dtypes.bfloat16)
    return hi, lo


def _weights() -> np.ndarray:
    """[128, 512] bf16 = [W0h | W1h | W0l | W1l].

    W0[c, p] = h[p-c], W1[c, p] = h[p-c+128], each split hi/lo."""
    h = _impulse_response(K_TAPS)
    c = np.arange(BLK)[:, None]
    p = np.arange(BLK)[None, :]
    k0 = p - c
    k1 = p - c + BLK
    w0 = np.where((k0 >= 0) & (k0 < K_TAPS), h[np.clip(k0, 0, K_TAPS - 1)], 0.0)
    w1 = np.where((k1 >= 0) & (k1 < K_TAPS), h[np.clip(k1, 0, K_TAPS - 1)], 0.0)
    w0h, w0l = _split_bf16(w0.astype(np.float32))
    w1h, w1l = _split_bf16(w1.astype(np.float32))
    return np.concatenate([w0h, w1h, w0l, w1l], axis=1)


_COMPILED = None


def _build():
    nc = bacc.Bacc("TRN2", target_bir_lowering=False, debug=False,
                   num_devices=N_CORES)
    f32 = mybir.dt.float32
    bf16 = mybir.dt.bfloat16
    xh = nc.declare_dram_parameter("xh", [BLK, ROWS_PER_CORE * COLS_IN], bf16,
                                   isOutput=False)
    xl = nc.declare_dram_parameter("xl", [BLK, ROWS_PER_CORE * COLS_IN], bf16,
                                   isOutput=False)
    w = nc.declare_dram_parameter("w", [BLK, 4 * BLK], bf16, isOutput=False)
    out = nc.declare_dram_parameter("out", [BLK, ROWS_PER_CORE * NBLK], f32,
                                    isOutput=True)

    with TileContext(nc) as tc:
        with (
            tc.tile_pool(name="wpool", bufs=1) as wpool,
            tc.tile_pool(name="xpool", bufs=3) as xpool,
            tc.tile_pool(name="opool", bufs=3) as opool,
            tc.tile_pool(name="pspool", bufs=4, space="PSUM") as pspool,
        ):
            wt = wpool.tile([BLK, 4 * BLK], bf16)
            nc.sync.dma_start(wt[:], w[:])
            w0h, w1h = wt[:, 0:BLK], wt[:, BLK:2 * BLK]
            w0l, w1l = wt[:, 2 * BLK:3 * BLK], wt[:, 3 * BLK:4 * BLK]
            # 4 rows per DMA: 10-20 KB contiguous per partition per transfer
            GRP = 4
            for g in range(ROWS_PER_CORE // GRP):
                xht = xpool.tile([BLK, GRP * COLS_IN], bf16, tag="xh")
                xlt = xpool.tile([BLK, GRP * COLS_IN], bf16, tag="xl")
                c0 = g * GRP * COLS_IN
                nc.sync.dma_start(xht[:], xh[:, c0:c0 + GRP * COLS_IN])
                nc.sync.dma_start(xlt[:], xl[:, c0:c0 + GRP * COLS_IN])
                ot = opool.tile([BLK, GRP * NBLK], f32)
                for i in range(GRP):
                    rb = i * COLS_IN
                    ob = i * NBLK
                    for j0, j1 in CHUNKS:
                        ps = pspool.tile([BLK, 512], f32)
                        n = j1 - j0
                        cur_h = xht[:, rb + 1 + j0:rb + 1 + j1]
                        prv_h = xht[:, rb + j0:rb + j1]
                        cur_l = xlt[:, rb + 1 + j0:rb + 1 + j1]
                        prv_l = xlt[:, rb + j0:rb + j1]
                        nc.tensor.matmul(ps[:, :n], w0h, cur_h, start=True, stop=False)
                        nc.tensor.matmul(ps[:, :n], w1h, prv_h, start=False, stop=False)
                        nc.tensor.matmul(ps[:, :n], w0l, cur_h, start=False, stop=False)
                        nc.tensor.matmul(ps[:, :n], w1l, prv_h, start=False, stop=False)
                        nc.tensor.matmul(ps[:, :n], w0h, cur_l, start=False, stop=False)
                        nc.tensor.matmul(ps[:, :n], w1h, prv_l, start=False, stop=True)
                        nc.vector.tensor_copy(ot[:, ob + j0:ob + j1], ps[:, :n])
                nc.sync.dma_start(out[:, g * GRP * NBLK:(g + 1) * GRP * NBLK],
                                  ot[:])
    nc.compile()
    return nc


def _get_compiled():
    global _COMPILED
    if _COMPILED is None:
        _COMPILED = _build()
    return _COMPILED


def kernel(clip: np.ndarray) -> np.ndarray:
    clip = np.ascontiguousarray(clip, dtype=np.float32)
    assert clip.shape == (B, T)
    w_np = _weights()
    in_maps = []
    for core in range(N_CORES):
        rows = clip[core * ROWS_PER_CORE:(core + 1) * ROWS_PER_CORE]
        # [16, 160000] -> [16, 1250, 128] -> per-row transpose [16, 128, 1250]
        blocks = rows.reshape(ROWS_PER_CORE, NBLK, BLK).transpose(0, 2, 1)
        xc = np.zeros((BLK, ROWS_PER_CORE, COLS_IN), dtype=np.float32)
        xc[:, :, 1:] = blocks.transpose(1, 0, 2)
        xc = xc.reshape(BLK, ROWS_PER_CORE * COLS_IN)
        xh_np, xl_np = _split_bf16(xc)
        in_maps.append({"xh": np.ascontiguousarray(xh_np),
                        "xl": np.ascontiguousarray(xl_np),
                        "w": w_np})

    nc = _get_compiled()
    res = bass_utils.run_bass_kernel_spmd(
        nc, in_maps, core_ids=list(range(N_CORES)))

    out = np.empty((B, T), dtype=np.float32)
    for core in range(N_CORES):
        yc = res.results[core]["out"]  # [128, 16*1250]
        yc = yc.reshape(BLK, ROWS_PER_CORE, NBLK).transpose(1, 2, 0)
        out[core * ROWS_PER_CORE:(core + 1) * ROWS_PER_CORE] = yc.reshape(
            ROWS_PER_CORE, T)
    return out
